# revision 17
# baseline (speedup 1.0000x reference)
"""Multi-LOD dense-grid trilinear interpolation (embedding_lookup) on 8 trn2 cores.

Strategy: data-parallel over points (8 cores x N/8 points; grids replicated).
Per (point, LOD, x-corner) we issue one per-partition-row indirect-DMA gather
fetching the contiguous y-span block rows[(x*r+y0)*r+z0 ... +(r+2) rows]
(covers the (y0,y1)x(z0,z1) corner patch at static offsets 0 and 4r within
the block). Each gather instruction serves 128 points (one offset per SBUF
partition) -- the only indirect-DMA form walrus/ucode executes correctly.
Weights + trilinear reduction run on the Vector engine; coordinate prep is
split between Scalar and Vector engines.

kernel(**inputs) takes FULL inputs, returns the FULL [N, 20] output.
"""

import math
import numpy as np

import concourse.bass as bass
import concourse.bacc as bacc
import concourse.mybir as mybir
import concourse.tile as tile
from concourse import bass_utils

P = 128
NUM_LODS = 5
FEAT = 4
LODS = [16, 32, 64, 128, 256]
N_PTS = 1_000_000
N_CORES = 8
OUT_D = NUM_LODS * FEAT  # 20

f32 = mybir.dt.float32
i32 = mybir.dt.int32

E_MAIN = 64  # point-columns per tile (tile = P*E points)
# per-LOD gather sub-batch (columns of 128 points per gd buffer fill)
ESUB = {16: 64, 32: 32, 64: 16, 128: 8, 256: 4}
# per-LOD gather shape: "yspan" = 2 descs/pt of (r+2) rows; "zpair" = 4 descs/pt of 2 rows
MODES = {16: "xspan", 32: "yspan", 64: "yspan", 128: "yspan", 256: "yspan"}
ESUB_ZPAIR = 64
ESUB_XSPAN = {16: 8}


def _v(t_ap: bass.AP, off_elems: int, dims) -> bass.AP:
    part = [list(t_ap.ap[0])[0], list(t_ap.ap[0])[1]]
    return bass.AP(
        t_ap.tensor,
        t_ap.offset + off_elems,
        [part] + [[int(s), int(c)] for s, c in dims],
    )


def padded_n_per_core(n: int = N_PTS) -> int:
    n_per_core = math.ceil(n / N_CORES)
    return P * E_MAIN * math.ceil(n_per_core / (P * E_MAIN))


def build_kernel(tc, out_ap, pts_ap, grid_aps, off_ap, n_padded):
    nc = tc.nc
    n_tiles = n_padded // (P * E_MAIN)
    E = E_MAIN

    with (
        tc.tile_pool(name="io", bufs=2) as io_pool,
        tc.tile_pool(name="sm", bufs=2) as sm_pool,
        tc.tile_pool(name="gd", bufs=2) as gd_pool,
        tc.tile_pool(name="cn", bufs=1) as cn_pool,
    ):
        off_t = cn_pool.tile([P, NUM_LODS * 6], f32, name="off_t")
        nc.sync.dma_start(off_t, off_ap)

        for ti in range(n_tiles):
            base = ti * P * E
            sfx = f"_{ti}"
            # pts layout: partition p, slot e  <-> point row base + e*128 + p
            pts_t = io_pool.tile([P, 3 * E], f32, tag="pts", name="pts" + sfx)
            src = bass.AP(
                pts_ap.tensor, base * 3, [[3, P], [3 * P, E], [1, 3]]
            )
            nc.sync.dma_start(pts_t, src)
            out_t = io_pool.tile([P, OUT_D * E], f32, tag="out", name="out" + sfx)

            for l, r in enumerate(LODS):
                lsfx = f"_{ti}_{l}"
                blk = (r + 2) * FEAT  # f32 elems per gathered block
                # ---- coords & robust floor ----
                c_t = sm_pool.tile([P, 3 * E], f32, tag="c", name="c" + lsfx)
                nc.scalar.mul(c_t, pts_t, float(r - 1))
                ii_t = sm_pool.tile([P, 3 * E], i32, tag="ii", name="ii" + lsfx)
                nc.vector.tensor_copy(ii_t, c_t)
                i0_t = sm_pool.tile([P, 3 * E], f32, tag="i0", name="i0" + lsfx)
                nc.vector.tensor_copy(i0_t, ii_t)
                cmp_t = sm_pool.tile([P, 3 * E], f32, tag="cmp", name="cmp" + lsfx)
                nc.vector.tensor_tensor(cmp_t, i0_t, c_t, mybir.AluOpType.is_gt)
                nc.vector.tensor_tensor(i0_t, i0_t, cmp_t, mybir.AluOpType.subtract)
                f_t = sm_pool.tile([P, 3 * E], f32, tag="f", name="f" + lsfx)
                nc.vector.tensor_tensor(f_t, c_t, i0_t, mybir.AluOpType.subtract)

                # ---- base index (x0*r + y0)*r + z0 ----
                t1_t = sm_pool.tile([P, E], f32, tag="t1", name="t1" + lsfx)
                nc.vector.scalar_tensor_tensor(
                    t1_t, _v(i0_t, 0, [[3, E]]), float(r), _v(i0_t, 1, [[3, E]]),
                    mybir.AluOpType.mult, mybir.AluOpType.add,
                )
                idx00_t = sm_pool.tile([P, E], f32, tag="idx00", name="idx00" + lsfx)
                nc.vector.scalar_tensor_tensor(
                    idx00_t, t1_t, float(r), _v(i0_t, 2, [[3, E]]),
                    mybir.AluOpType.mult, mybir.AluOpType.add,
                )
                # corner base indices: yspan -> [e][i] = idx00 + i*r^2 (2)
                #                      zpair -> [e][c] = idx00 + {0,r,r^2,r^2+r} (4)
                mode = MODES[r]
                nco = {"xspan": 1, "yspan": 2, "zpair": 4}[mode]
                ooff = 6 * l + (2 if mode == "zpair" else 0)
                idx2f_t = sm_pool.tile([P, 4 * E], f32, tag="idx2f", name="idx2f" + lsfx)
                nc.vector.tensor_tensor(
                    _v(idx2f_t, 0, [[nco, E], [1, nco]]),
                    _v(idx00_t, 0, [[1, E], [0, nco]]),
                    _v(off_t, ooff, [[0, E], [1, nco]]),
                    mybir.AluOpType.add,
                )
                idx2_t = sm_pool.tile([P, 4 * E], i32, tag="idx2", name="idx2" + lsfx)
                nc.vector.tensor_copy(
                    _v(idx2_t, 0, [[nco, E], [1, nco]]),
                    _v(idx2f_t, 0, [[nco, E], [1, nco]]),
                )

                # ---- weights: w8[e, i*4 + j*2 + zb] ----
                g1_t = sm_pool.tile([P, 3 * E], f32, tag="g1", name="g1" + lsfx)
                nc.scalar.activation(
                    g1_t, f_t, mybir.ActivationFunctionType.Copy, bias=1.0, scale=-1.0
                )
                # xy2[e*4 + 2*axis + sel]: axis0 x: [gx, fx], axis1 y: [gy, fy]
                xy2_t = sm_pool.tile([P, 4 * E], f32, tag="xy2", name="xy2" + lsfx)
                nc.vector.tensor_copy(
                    _v(xy2_t, 0, [[4, E], [2, 2]]), _v(g1_t, 0, [[3, E], [1, 2]])
                )
                nc.vector.tensor_copy(
                    _v(xy2_t, 1, [[4, E], [2, 2]]), _v(f_t, 0, [[3, E], [1, 2]])
                )
                # w4[e, i*2+j] = xw[i] * yw[j]
                w4_t = sm_pool.tile([P, 4 * E], f32, tag="w4", name="w4" + lsfx)
                nc.vector.tensor_tensor(
                    _v(w4_t, 0, [[4, E], [2, 2], [1, 2]]),
                    _v(xy2_t, 0, [[4, E], [1, 2], [0, 2]]),
                    _v(xy2_t, 2, [[4, E], [0, 2], [1, 2]]),
                    mybir.AluOpType.mult,
                )
                # w8[e, c*2+zb] = w4[c] * (zb ? fz : gz)
                w8_t = sm_pool.tile([P, 8 * E], f32, tag="w8", name="w8" + lsfx)
                nc.vector.tensor_tensor(
                    _v(w8_t, 0, [[8, E], [2, 4]]),
                    _v(w4_t, 0, [[4, E], [1, 4]]),
                    _v(g1_t, 2, [[3, E], [0, 4]]),
                    mybir.AluOpType.mult,
                )
                nc.vector.tensor_tensor(
                    _v(w8_t, 1, [[8, E], [2, 4]]),
                    _v(w4_t, 0, [[4, E], [1, 4]]),
                    _v(f_t, 2, [[3, E], [0, 4]]),
                    mybir.AluOpType.mult,
                )

                # ---- gather + weighted reduce, in sub-batches of Es columns ----
                if mode == "xspan":
                    Es = ESUB_XSPAN[r]
                elif mode == "yspan":
                    Es = ESUB[r]
                else:
                    Es = ESUB_ZPAIR
                blk_x = (r * r + r + 2) * FEAT
                for e0 in range(0, E, Es):
                    ssfx = f"{lsfx}_{e0}"
                    if mode == "xspan":
                        gd_t = gd_pool.tile(
                            [P, Es * blk_x], f32, tag="gd", name="gd" + ssfx
                        )
                        for ee in range(Es):
                            col = e0 + ee
                            nc.gpsimd.indirect_dma_start(
                                out=gd_t[:, ee * blk_x : (ee + 1) * blk_x],
                                out_offset=None,
                                in_=grid_aps[l],
                                in_offset=bass.IndirectOffsetOnAxis(
                                    ap=idx2_t[:, col : col + 1], axis=0
                                ),
                            )
                    elif mode == "yspan":
                        gd_t = gd_pool.tile(
                            [P, 2 * Es * blk], f32, tag="gd", name="gd" + ssfx
                        )
                        for ee in range(Es):
                            col = e0 + ee
                            for i in range(2):
                                nc.gpsimd.indirect_dma_start(
                                    out=gd_t[:, (2 * ee + i) * blk : (2 * ee + i + 1) * blk],
                                    out_offset=None,
                                    in_=grid_aps[l],
                                    in_offset=bass.IndirectOffsetOnAxis(
                                        ap=idx2_t[:, 2 * col + i : 2 * col + i + 1], axis=0
                                    ),
                                )
                    else:
                        gd_t = gd_pool.tile(
                            [P, 4 * Es * 8], f32, tag="gd", name="gd" + ssfx
                        )
                        for ee in range(Es):
                            col = e0 + ee
                            for c in range(4):
                                nc.gpsimd.indirect_dma_start(
                                    out=gd_t[:, (4 * ee + c) * 8 : (4 * ee + c + 1) * 8],
                                    out_offset=None,
                                    in_=grid_aps[l],
                                    in_offset=bass.IndirectOffsetOnAxis(
                                        ap=idx2_t[:, 4 * col + c : 4 * col + c + 1], axis=0
                                    ),
                                )
                    # weighted terms gw[e, term, f]; term = i*4 + j*2 + zb.
                    # One mult per (i,j) corner covers both zb (z-pair contiguous).
                    gw_t = sm_pool.tile([P, 8 * Es * FEAT], f32, tag="gw", name="gw" + ssfx)
                    for i in range(2):
                        for j in range(2):
                            if mode == "xspan":
                                in0 = _v(gd_t, (i * r * r + j * r) * 4, [[blk_x, Es], [1, 8]])
                            elif mode == "yspan":
                                in0 = _v(gd_t, i * blk + j * 4 * r, [[2 * blk, Es], [1, 8]])
                            else:
                                in0 = _v(gd_t, (i * 2 + j) * 8, [[32, Es], [1, 8]])
                            nc.vector.tensor_tensor(
                                _v(gw_t, (i * 4 + j * 2) * 4, [[32, Es], [1, 8]]),
                                in0,
                                _v(w8_t, 8 * e0 + i * 4 + j * 2, [[8, Es], [1, 2], [0, 4]]),
                                mybir.AluOpType.mult,
                            )
                    s1_t = sm_pool.tile([P, 4 * Es * FEAT], f32, tag="s1", name="s1" + ssfx)
                    nc.vector.tensor_tensor(
                        _v(s1_t, 0, [[16, Es], [1, 16]]),
                        _v(gw_t, 0, [[32, Es], [1, 16]]),
                        _v(gw_t, 16, [[32, Es], [1, 16]]),
                        mybir.AluOpType.add,
                    )
                    s2_t = sm_pool.tile([P, 2 * Es * FEAT], f32, tag="s2", name="s2" + ssfx)
                    nc.vector.tensor_tensor(
                        _v(s2_t, 0, [[8, Es], [1, 8]]),
                        _v(s1_t, 0, [[16, Es], [1, 8]]),
                        _v(s1_t, 8, [[16, Es], [1, 8]]),
                        mybir.AluOpType.add,
                    )
                    nc.vector.tensor_tensor(
                        _v(out_t, OUT_D * e0 + 4 * l, [[OUT_D, Es], [1, 4]]),
                        _v(s2_t, 0, [[8, Es], [1, 4]]),
                        _v(s2_t, 4, [[8, Es], [1, 4]]),
                        mybir.AluOpType.add,
                    )

            dst = bass.AP(
                out_ap.tensor, base * OUT_D, [[OUT_D, P], [OUT_D * P, E], [1, OUT_D]]
            )
            nc.sync.dma_start(dst, out_t)


def _make_off_const() -> np.ndarray:
    row = np.zeros(NUM_LODS * 6, dtype=np.float32)
    for l, r in enumerate(LODS):
        row[6 * l : 6 * l + 6] = [0.0, float(r * r), 0.0, float(r), float(r * r), float(r * r + r)]
    return np.tile(row[None, :], (P, 1))


_COMPILED = {}


def _get_compiled(n_padded: int = None):
    if n_padded is None:
        n_padded = padded_n_per_core()
    if n_padded in _COMPILED:
        return _COMPILED[n_padded]
    nc = bacc.Bacc("TRN2", debug=False, enable_asserts=False)
    pts_ap = nc.dram_tensor("pts", [n_padded, 3], f32, kind="ExternalInput").ap()
    grid_aps = [
        nc.dram_tensor(f"grid{l}", [LODS[l] ** 3, FEAT], f32, kind="ExternalInput").ap()
        for l in range(NUM_LODS)
    ]
    off_ap = nc.dram_tensor("offs", [P, NUM_LODS * 6], f32, kind="ExternalInput").ap()
    out_ap = nc.dram_tensor("out", [n_padded, OUT_D], f32, kind="ExternalOutput").ap()
    with tile.TileContext(nc) as tc:
        build_kernel(tc, out_ap, pts_ap, grid_aps, off_ap, n_padded)
    nc.compile()
    _COMPILED[n_padded] = nc
    return nc


def kernel(pts, grid0, grid1, grid2, grid3, grid4, _trace=False, _tmpdir=None):
    pts = np.ascontiguousarray(np.asarray(pts, dtype=np.float32))
    grids = [
        np.ascontiguousarray(np.asarray(g, dtype=np.float32))
        for g in (grid0, grid1, grid2, grid3, grid4)
    ]
    n = pts.shape[0]
    n_per_core = math.ceil(n / N_CORES)
    n_padded = padded_n_per_core(n)
    offs = _make_off_const()

    nc = _get_compiled(n_padded)
    in_maps = []
    for c in range(N_CORES):
        lo = c * n_per_core
        hi = min(n, (c + 1) * n_per_core)
        chunk = np.zeros((n_padded, 3), dtype=np.float32)
        chunk[: hi - lo] = pts[lo:hi]
        m = {"pts": chunk, "offs": offs}
        for l in range(NUM_LODS):
            m[f"grid{l}"] = grids[l]
        in_maps.append(m)

    res = bass_utils.run_bass_kernel_spmd(
        nc, in_maps, core_ids=list(range(N_CORES)), trace=_trace, tmpdir=_tmpdir
    )
    out = np.empty((n, OUT_D), dtype=np.float32)
    for c in range(N_CORES):
        lo = c * n_per_core
        hi = min(n, (c + 1) * n_per_core)
        core_out = res.results[c]["out"]
        # invert the (e p) layout: row base+e*128+p holds point base+e*128+p -- identity
        out[lo:hi] = core_out[: hi - lo]
    kernel.last_results = res
    return out


# revision 18
# speedup vs baseline: 1.1509x; 1.1509x over previous
"""Multi-LOD dense-grid trilinear interpolation (embedding_lookup) on 8 trn2 cores.

Strategy: data-parallel over points (8 cores x N/8 points; grids replicated).
Per (point, LOD, x-corner) we issue one per-partition-row indirect-DMA gather
fetching the contiguous y-span block rows[(x*r+y0)*r+z0 ... +(r+2) rows]
(covers the (y0,y1)x(z0,z1) corner patch at static offsets 0 and 4r within
the block). Each gather instruction serves 128 points (one offset per SBUF
partition) -- the only indirect-DMA form walrus/ucode executes correctly.
Weights + trilinear reduction run on the Vector engine; coordinate prep is
split between Scalar and Vector engines.

kernel(**inputs) takes FULL inputs, returns the FULL [N, 20] output.
"""

import math
import numpy as np

import concourse.bass as bass
import concourse.bacc as bacc
import concourse.mybir as mybir
import concourse.tile as tile
from concourse import bass_utils

P = 128
NUM_LODS = 5
FEAT = 4
LODS = [16, 32, 64, 128, 256]
N_PTS = 1_000_000
N_CORES = 8
OUT_D = NUM_LODS * FEAT  # 20

f32 = mybir.dt.float32
i32 = mybir.dt.int32

E_MAIN = 64  # point-columns per tile (tile = P*E points)
# per-LOD gather sub-batch (columns of 128 points per gd buffer fill)
ESUB = {16: 64, 32: 32, 64: 16, 128: 8, 256: 4}
# per-LOD gather shape: "yspan" = 2 descs/pt of (r+2) rows; "zpair" = 4 descs/pt of 2 rows
MODES = {16: "xspan", 32: "yspan", 64: "yspan", 128: "yspan", 256: "yspan"}
ESUB_ZPAIR = 64
ESUB_XSPAN = {16: 8}
DMA_SCRATCH = 16384  # SWDGE descriptor-ring carveout (bytes)


def _v(t_ap: bass.AP, off_elems: int, dims) -> bass.AP:
    part = [list(t_ap.ap[0])[0], list(t_ap.ap[0])[1]]
    return bass.AP(
        t_ap.tensor,
        t_ap.offset + off_elems,
        [part] + [[int(s), int(c)] for s, c in dims],
    )


def padded_n_per_core(n: int = N_PTS) -> int:
    n_per_core = math.ceil(n / N_CORES)
    return P * E_MAIN * math.ceil(n_per_core / (P * E_MAIN))


def build_kernel(tc, out_ap, pts_ap, grid_aps, off_ap, n_padded):
    nc = tc.nc
    n_tiles = n_padded // (P * E_MAIN)
    E = E_MAIN

    with (
        tc.tile_pool(name="io", bufs=2) as io_pool,
        tc.tile_pool(name="sm", bufs=2) as sm_pool,
        tc.tile_pool(name="gd", bufs=2) as gd_pool,
        tc.tile_pool(name="cn", bufs=1) as cn_pool,
    ):
        off_t = cn_pool.tile([P, NUM_LODS * 6], f32, name="off_t")
        nc.sync.dma_start(off_t, off_ap)

        for ti in range(n_tiles):
            base = ti * P * E
            sfx = f"_{ti}"
            # pts layout: partition p, slot e  <-> point row base + e*128 + p
            pts_t = io_pool.tile([P, 3 * E], f32, tag="pts", name="pts" + sfx)
            src = bass.AP(
                pts_ap.tensor, base * 3, [[3, P], [3 * P, E], [1, 3]]
            )
            nc.sync.dma_start(pts_t, src)
            out_t = io_pool.tile([P, OUT_D * E], f32, tag="out", name="out" + sfx)

            for l, r in enumerate(LODS):
                lsfx = f"_{ti}_{l}"
                blk = (r + 2) * FEAT  # f32 elems per gathered block
                # ---- coords & robust floor ----
                c_t = sm_pool.tile([P, 3 * E], f32, tag="c", name="c" + lsfx)
                nc.scalar.mul(c_t, pts_t, float(r - 1))
                ii_t = sm_pool.tile([P, 3 * E], i32, tag="ii", name="ii" + lsfx)
                nc.vector.tensor_copy(ii_t, c_t)
                i0_t = sm_pool.tile([P, 3 * E], f32, tag="i0", name="i0" + lsfx)
                nc.vector.tensor_copy(i0_t, ii_t)
                cmp_t = sm_pool.tile([P, 3 * E], f32, tag="cmp", name="cmp" + lsfx)
                nc.vector.tensor_tensor(cmp_t, i0_t, c_t, mybir.AluOpType.is_gt)
                nc.vector.tensor_tensor(i0_t, i0_t, cmp_t, mybir.AluOpType.subtract)
                f_t = sm_pool.tile([P, 3 * E], f32, tag="f", name="f" + lsfx)
                nc.vector.tensor_tensor(f_t, c_t, i0_t, mybir.AluOpType.subtract)

                # ---- base index (x0*r + y0)*r + z0 ----
                t1_t = sm_pool.tile([P, E], f32, tag="t1", name="t1" + lsfx)
                nc.vector.scalar_tensor_tensor(
                    t1_t, _v(i0_t, 0, [[3, E]]), float(r), _v(i0_t, 1, [[3, E]]),
                    mybir.AluOpType.mult, mybir.AluOpType.add,
                )
                idx00_t = sm_pool.tile([P, E], f32, tag="idx00", name="idx00" + lsfx)
                nc.vector.scalar_tensor_tensor(
                    idx00_t, t1_t, float(r), _v(i0_t, 2, [[3, E]]),
                    mybir.AluOpType.mult, mybir.AluOpType.add,
                )
                # corner base indices: yspan -> [e][i] = idx00 + i*r^2 (2)
                #                      zpair -> [e][c] = idx00 + {0,r,r^2,r^2+r} (4)
                mode = MODES[r]
                nco = {"xspan": 1, "yspan": 2, "zpair": 4}[mode]
                ooff = 6 * l + (2 if mode == "zpair" else 0)
                idx2f_t = sm_pool.tile([P, 4 * E], f32, tag="idx2f", name="idx2f" + lsfx)
                nc.vector.tensor_tensor(
                    _v(idx2f_t, 0, [[nco, E], [1, nco]]),
                    _v(idx00_t, 0, [[1, E], [0, nco]]),
                    _v(off_t, ooff, [[0, E], [1, nco]]),
                    mybir.AluOpType.add,
                )
                idx2_t = sm_pool.tile([P, 4 * E], i32, tag="idx2", name="idx2" + lsfx)
                nc.vector.tensor_copy(
                    _v(idx2_t, 0, [[nco, E], [1, nco]]),
                    _v(idx2f_t, 0, [[nco, E], [1, nco]]),
                )

                # ---- weights: w8[e, i*4 + j*2 + zb] ----
                g1_t = sm_pool.tile([P, 3 * E], f32, tag="g1", name="g1" + lsfx)
                nc.scalar.activation(
                    g1_t, f_t, mybir.ActivationFunctionType.Copy, bias=1.0, scale=-1.0
                )
                # xy2[e*4 + 2*axis + sel]: axis0 x: [gx, fx], axis1 y: [gy, fy]
                xy2_t = sm_pool.tile([P, 4 * E], f32, tag="xy2", name="xy2" + lsfx)
                nc.vector.tensor_copy(
                    _v(xy2_t, 0, [[4, E], [2, 2]]), _v(g1_t, 0, [[3, E], [1, 2]])
                )
                nc.vector.tensor_copy(
                    _v(xy2_t, 1, [[4, E], [2, 2]]), _v(f_t, 0, [[3, E], [1, 2]])
                )
                # w4[e, i*2+j] = xw[i] * yw[j]
                w4_t = sm_pool.tile([P, 4 * E], f32, tag="w4", name="w4" + lsfx)
                nc.vector.tensor_tensor(
                    _v(w4_t, 0, [[4, E], [2, 2], [1, 2]]),
                    _v(xy2_t, 0, [[4, E], [1, 2], [0, 2]]),
                    _v(xy2_t, 2, [[4, E], [0, 2], [1, 2]]),
                    mybir.AluOpType.mult,
                )
                # w8[e, c*2+zb] = w4[c] * (zb ? fz : gz)
                w8_t = sm_pool.tile([P, 8 * E], f32, tag="w8", name="w8" + lsfx)
                nc.vector.tensor_tensor(
                    _v(w8_t, 0, [[8, E], [2, 4]]),
                    _v(w4_t, 0, [[4, E], [1, 4]]),
                    _v(g1_t, 2, [[3, E], [0, 4]]),
                    mybir.AluOpType.mult,
                )
                nc.vector.tensor_tensor(
                    _v(w8_t, 1, [[8, E], [2, 4]]),
                    _v(w4_t, 0, [[4, E], [1, 4]]),
                    _v(f_t, 2, [[3, E], [0, 4]]),
                    mybir.AluOpType.mult,
                )

                # ---- gather + weighted reduce, in sub-batches of Es columns ----
                if mode == "xspan":
                    Es = ESUB_XSPAN[r]
                elif mode == "yspan":
                    Es = ESUB[r]
                else:
                    Es = ESUB_ZPAIR
                blk_x = (r * r + r + 2) * FEAT
                for e0 in range(0, E, Es):
                    ssfx = f"{lsfx}_{e0}"
                    if mode == "xspan":
                        gd_t = gd_pool.tile(
                            [P, Es * blk_x], f32, tag="gd", name="gd" + ssfx
                        )
                        for ee in range(Es):
                            col = e0 + ee
                            nc.gpsimd.indirect_dma_start(
                                out=gd_t[:, ee * blk_x : (ee + 1) * blk_x],
                                out_offset=None,
                                in_=grid_aps[l],
                                in_offset=bass.IndirectOffsetOnAxis(
                                    ap=idx2_t[:, col : col + 1], axis=0
                                ),
                            )
                    elif mode == "yspan":
                        gd_t = gd_pool.tile(
                            [P, 2 * Es * blk], f32, tag="gd", name="gd" + ssfx
                        )
                        for ee in range(Es):
                            col = e0 + ee
                            for i in range(2):
                                nc.gpsimd.indirect_dma_start(
                                    out=gd_t[:, (2 * ee + i) * blk : (2 * ee + i + 1) * blk],
                                    out_offset=None,
                                    in_=grid_aps[l],
                                    in_offset=bass.IndirectOffsetOnAxis(
                                        ap=idx2_t[:, 2 * col + i : 2 * col + i + 1], axis=0
                                    ),
                                )
                    else:
                        gd_t = gd_pool.tile(
                            [P, 4 * Es * 8], f32, tag="gd", name="gd" + ssfx
                        )
                        for ee in range(Es):
                            col = e0 + ee
                            for c in range(4):
                                nc.gpsimd.indirect_dma_start(
                                    out=gd_t[:, (4 * ee + c) * 8 : (4 * ee + c + 1) * 8],
                                    out_offset=None,
                                    in_=grid_aps[l],
                                    in_offset=bass.IndirectOffsetOnAxis(
                                        ap=idx2_t[:, 4 * col + c : 4 * col + c + 1], axis=0
                                    ),
                                )
                    # weighted terms gw[e, term, f]; term = i*4 + j*2 + zb.
                    # One mult per (i,j) corner covers both zb (z-pair contiguous).
                    gw_t = sm_pool.tile([P, 8 * Es * FEAT], f32, tag="gw", name="gw" + ssfx)
                    for i in range(2):
                        for j in range(2):
                            if mode == "xspan":
                                in0 = _v(gd_t, (i * r * r + j * r) * 4, [[blk_x, Es], [1, 8]])
                            elif mode == "yspan":
                                in0 = _v(gd_t, i * blk + j * 4 * r, [[2 * blk, Es], [1, 8]])
                            else:
                                in0 = _v(gd_t, (i * 2 + j) * 8, [[32, Es], [1, 8]])
                            nc.vector.tensor_tensor(
                                _v(gw_t, (i * 4 + j * 2) * 4, [[32, Es], [1, 8]]),
                                in0,
                                _v(w8_t, 8 * e0 + i * 4 + j * 2, [[8, Es], [1, 2], [0, 4]]),
                                mybir.AluOpType.mult,
                            )
                    s1_t = sm_pool.tile([P, 4 * Es * FEAT], f32, tag="s1", name="s1" + ssfx)
                    nc.vector.tensor_tensor(
                        _v(s1_t, 0, [[16, Es], [1, 16]]),
                        _v(gw_t, 0, [[32, Es], [1, 16]]),
                        _v(gw_t, 16, [[32, Es], [1, 16]]),
                        mybir.AluOpType.add,
                    )
                    s2_t = sm_pool.tile([P, 2 * Es * FEAT], f32, tag="s2", name="s2" + ssfx)
                    nc.vector.tensor_tensor(
                        _v(s2_t, 0, [[8, Es], [1, 8]]),
                        _v(s1_t, 0, [[16, Es], [1, 8]]),
                        _v(s1_t, 8, [[16, Es], [1, 8]]),
                        mybir.AluOpType.add,
                    )
                    nc.vector.tensor_tensor(
                        _v(out_t, OUT_D * e0 + 4 * l, [[OUT_D, Es], [1, 4]]),
                        _v(s2_t, 0, [[8, Es], [1, 4]]),
                        _v(s2_t, 4, [[8, Es], [1, 4]]),
                        mybir.AluOpType.add,
                    )

            dst = bass.AP(
                out_ap.tensor, base * OUT_D, [[OUT_D, P], [OUT_D * P, E], [1, OUT_D]]
            )
            nc.sync.dma_start(dst, out_t)


def _make_off_const() -> np.ndarray:
    row = np.zeros(NUM_LODS * 6, dtype=np.float32)
    for l, r in enumerate(LODS):
        row[6 * l : 6 * l + 6] = [0.0, float(r * r), 0.0, float(r), float(r * r), float(r * r + r)]
    return np.tile(row[None, :], (P, 1))


_COMPILED = {}


def _get_compiled(n_padded: int = None):
    if n_padded is None:
        n_padded = padded_n_per_core()
    if n_padded in _COMPILED:
        return _COMPILED[n_padded]
    nc = bacc.Bacc(
        "TRN2", debug=False, enable_asserts=False,
        dynamic_dma_scratch_size=DMA_SCRATCH,
    )
    pts_ap = nc.dram_tensor("pts", [n_padded, 3], f32, kind="ExternalInput").ap()
    grid_aps = [
        nc.dram_tensor(f"grid{l}", [LODS[l] ** 3, FEAT], f32, kind="ExternalInput").ap()
        for l in range(NUM_LODS)
    ]
    off_ap = nc.dram_tensor("offs", [P, NUM_LODS * 6], f32, kind="ExternalInput").ap()
    out_ap = nc.dram_tensor("out", [n_padded, OUT_D], f32, kind="ExternalOutput").ap()
    with tile.TileContext(nc) as tc:
        build_kernel(tc, out_ap, pts_ap, grid_aps, off_ap, n_padded)
    nc.compile()
    _COMPILED[n_padded] = nc
    return nc


def kernel(pts, grid0, grid1, grid2, grid3, grid4, _trace=False, _tmpdir=None):
    pts = np.ascontiguousarray(np.asarray(pts, dtype=np.float32))
    grids = [
        np.ascontiguousarray(np.asarray(g, dtype=np.float32))
        for g in (grid0, grid1, grid2, grid3, grid4)
    ]
    n = pts.shape[0]
    n_per_core = math.ceil(n / N_CORES)
    n_padded = padded_n_per_core(n)
    offs = _make_off_const()

    nc = _get_compiled(n_padded)
    in_maps = []
    for c in range(N_CORES):
        lo = c * n_per_core
        hi = min(n, (c + 1) * n_per_core)
        chunk = np.zeros((n_padded, 3), dtype=np.float32)
        chunk[: hi - lo] = pts[lo:hi]
        m = {"pts": chunk, "offs": offs}
        for l in range(NUM_LODS):
            m[f"grid{l}"] = grids[l]
        in_maps.append(m)

    res = bass_utils.run_bass_kernel_spmd(
        nc, in_maps, core_ids=list(range(N_CORES)), trace=_trace, tmpdir=_tmpdir
    )
    out = np.empty((n, OUT_D), dtype=np.float32)
    for c in range(N_CORES):
        lo = c * n_per_core
        hi = min(n, (c + 1) * n_per_core)
        core_out = res.results[c]["out"]
        # invert the (e p) layout: row base+e*128+p holds point base+e*128+p -- identity
        out[lo:hi] = core_out[: hi - lo]
    kernel.last_results = res
    return out
